# revision 38
# baseline (speedup 1.0000x reference)
"""Distributed multi-head self-attention for Trainium2 (8 NeuronCores).

Problem: b=4, n=2048, dim=1024, heads=16, dim_head=64.
  q = x@Wq; k,v = split(x@Wkv, 2); out = softmax(q k^T / 8) v; y = out@Wout + bout

Sharding: core c <-> (batch b=c//2, head-group g=c%2). Each core computes
q/k/v + attention for its batch's 8 heads (tensor-parallel columns of
Wq/Wkv). The pair (b,0)/(b,1) AllGathers the transposed bf16 attention
outputs (per head-pair, overlapped with attention compute; the last pair
streams per-i-chunk), then each core runs the output projection with the
full Wout over ITS HALF of the sequence (selected from the gathered buffer
with per-core one-hot mask inputs, since the SPMD graph is identical on all
cores). Core 2b+g emits out rows [1024g : 1024(g+1)] of batch b; the host
reassembles [4, 2048, 1024].

Host-side prep: x is pre-transposed to xT [dim, n] and converted to bf16,
weights are sliced per-core and converted to bf16 — phase 0 is pure DMA
(no on-device casts or PE transposes). A dummy 2-rank AllGather and a tiny
exp activation are issued at t=0 so the one-time collective rendezvous
(~100us measured) and the ACT table load (~2.7us) overlap with the
projection phase instead of stalling the attention pipeline.

TensorEngine math is bf16 with f32 PSUM accumulation. Softmax skips
max-subtraction (scaled scores are ~N(0,1)); exp runs on the scalar engine
(PSUM f32 in -> bf16 SBUF out, scale fused). Denominators come from a ones
column appended to v; the PSUM accumulator is released with two fast copies
and normalization (reciprocal_approx_fast + partition_broadcast + in-place
multiply) runs lazily off the critical path. Score matmuls (K=64) run two
heads concurrently via tile_position row groups. q/k projections for pair
p+1 are emitted after attention(p) and v projections inside attention(0)'s
first column loop, so the TensorEngine fills its slack while attention is
ACT(exp)-bound.
"""

from contextlib import ExitStack

import numpy as np
import ml_dtypes

import concourse.mybir as mybir
import concourse.tile as tile
from concourse import bacc, bass_utils

N_CORES = 8
B, N, D = 4, 2048, 1024
GH = 8          # heads per core
DH = 64
IN = GH * DH    # 512 inner dims per core
SCALE = DH ** -0.5
PT = 128
KD = D // PT    # 8 dim tiles
MS = N // PT    # 16 seq tiles
MI = 4          # head pairs per core
NH = N // 2     # out rows per core
F32 = mybir.dt.float32
BF16 = mybir.dt.bfloat16
RG = [[0, 1], [2, 3], [4, 5], [6, 7]]

_COMPILED = None


def build():
    nc = bacc.Bacc("TRN2", target_bir_lowering=False, debug=False, num_devices=N_CORES)

    xt_ext = nc.dram_tensor("xt", [4, KD, PT, 512], BF16, kind="ExternalInput")
    wq_ext = nc.dram_tensor("wq", [D, IN], BF16, kind="ExternalInput")
    wk_ext = nc.dram_tensor("wk", [D, IN], BF16, kind="ExternalInput")
    wv_ext = nc.dram_tensor("wv", [D, IN], BF16, kind="ExternalInput")
    wout_ext = nc.dram_tensor("wout", [D, D], BF16, kind="ExternalInput")
    bout_ext = nc.dram_tensor("bout", [D], F32, kind="ExternalInput")
    sel_ext = nc.dram_tensor("sel", [1, 2], F32, kind="ExternalInput")
    out_ext = nc.dram_tensor("out", [NH, D], F32, kind="ExternalOutput")

    with tile.TileContext(nc) as tc:
        with (
            tc.tile_pool(name="const", bufs=1) as constp,
            tc.tile_pool(name="wpool", bufs=1) as wpool,
            tc.tile_pool(name="qkv", bufs=1) as qkv,
            tc.tile_pool(name="attout", bufs=1) as attoutp,
            tc.tile_pool(name="dram", bufs=1, space="DRAM") as dram,
        ):
            # ---- t=0 warmups: collective rendezvous + exp table load ----
            wa_in = dram.tile([1, 16], BF16, name="wa_in")
            wa_out = dram.tile([2, 16], BF16, name="wa_out")
            wa_sb = constp.tile([1, 16], BF16)
            nc.gpsimd.memset(wa_sb[:], 0.0)
            nc.sync.dma_start(wa_in[:], wa_sb[:])
            nc.gpsimd.collective_compute(
                "AllGather", mybir.AluOpType.bypass,
                replica_groups=RG,
                ins=[wa_in.opt()], outs=[wa_out.opt()],
            )
            wex_in = constp.tile([1, 16], F32)
            nc.gpsimd.memset(wex_in[:], 0.0)
            wex_out = constp.tile([1, 16], BF16)
            nc.scalar.activation(
                wex_out[:], wex_in[:], mybir.ActivationFunctionType.Exp, scale=1.0
            )

            bias_row = constp.tile([1, D], F32)
            nc.sync.dma_start(bias_row[:], bout_ext[None, :])
            bias_bc = [constp.tile([PT, 512], F32, name=f"bias_bc{nn}")
                       for nn in range(2)]
            for nn in range(2):
                nc.gpsimd.partition_broadcast(
                    bias_bc[nn][:], bias_row[:, nn * 512:(nn + 1) * 512]
                )
            sel_row = constp.tile([1, 2], F32)
            nc.sync.dma_start(sel_row[:], sel_ext[:])
            s0_bc = constp.tile([PT, 1], F32)
            s1_bc = constp.tile([PT, 1], F32)
            nc.gpsimd.partition_broadcast(s0_bc[:], sel_row[:, 0:1])
            nc.gpsimd.partition_broadcast(s1_bc[:], sel_row[:, 1:2])

            wq_bf = [wpool.tile([PT, IN], BF16, name=f"wq_bf{k}") for k in range(KD)]
            wk_bf = [wpool.tile([PT, IN], BF16, name=f"wk_bf{k}") for k in range(KD)]
            wo_bf = [wpool.tile([PT, D], BF16, name=f"wo_bf{k}") for k in range(KD)]
            xT = [wpool.tile([PT, N], BF16, name=f"xT{k}") for k in range(KD)]

            qT = [qkv.tile([PT, N], BF16, name=f"qT{m}") for m in range(MI)]
            kT = [qkv.tile([PT, N], BF16, name=f"kT{m}") for m in range(MI)]
            vsb = [qkv.tile([PT, GH, 66], BF16, name=f"v{s}") for s in range(MS)]

            attoutT = [attoutp.tile([PT, N], BF16, name=f"attoutT{p}") for p in range(MI)]
            # after AG(p) the attoutT data is snapshotted to DRAM; reuse the
            # tile halves for the mask-selected gathered k-tiles kk=p
            # (cols 0:NH) and kk=p+MI (cols NH:N)
            attThalf = [
                attoutT[k % MI][:, (k // MI) * NH:(k // MI + 1) * NH]
                for k in range(KD)
            ]
            ag_in = [dram.tile([PT, N], BF16, name=f"ag_in{p}") for p in range(MI - 1)]
            ag_out = [dram.tile([2 * PT, N], BF16, name=f"ag_out{p}") for p in range(MI - 1)]
            ag_chunk = [dram.tile([2 * PT, 512], BF16, name=f"ag_chunk{i}") for i in range(4)]
            ag_cin = [dram.tile([PT, 512], BF16, name=f"ag_cin{i}") for i in range(4)]

            psum_stack = ExitStack()
            with (
                tc.tile_pool(name="attn", bufs=5) as attnp,
                tc.tile_pool(name="fin", bufs=2) as finp,
                tc.tile_pool(name="agst", bufs=1) as agst,
            ):
                # the PE's lead over exp is capped at 2 steps (~2.3us) by
                # psS double-buffering, so every hook must stay under that
                # much PE work or the ACT engine bubbles
                psP = psum_stack.enter_context(
                    tc.tile_pool(name="psP", bufs=2, space="PSUM"))
                psS = psum_stack.enter_context(
                    tc.tile_pool(name="psS", bufs=2, space="PSUM"))
                psO = psum_stack.enter_context(
                    tc.tile_pool(name="psO", bufs=2, space="PSUM"))

                oacc_stack = ExitStack()
                oaccp = oacc_stack.enter_context(
                    tc.tile_pool(name="oaccp", bufs=1))
                oacc = [
                    [oaccp.tile([PT, 512], F32, name=f"oacc{m}_{nn}")
                     for nn in range(2)]
                    for m in range(NH // PT)
                ]

                wv_stack = ExitStack()
                wvp = wv_stack.enter_context(tc.tile_pool(name="wvp", bufs=1))
                wv_bf = [wvp.tile([PT, IN], BF16, name=f"wv_bf{k}")
                         for k in range(KD)]

                # ones columns for the softmax denominators (no data deps)
                for s in range(MS):
                    nc.gpsimd.memset(vsb[s][:, :, 64:65], 1.0)

                # ---- phase 0 DMAs. Triggers cost ~0.6us each on an engine
                # queue, so spread them across the engines that are idle
                # during the load (xt is host-tiled: contiguous blocks) ----
                for k in range(KD):
                    nc.scalar.dma_start(wq_bf[k][:], wq_ext[k * PT:(k + 1) * PT, :])
                for k in range(KD):
                    nc.sync.dma_start(wk_bf[k][:], wk_ext[k * PT:(k + 1) * PT, :])
                for k in range(KD):
                    nc.sync.dma_start(xT[k][:, 0:512], xt_ext[0, k])
                for k in range(KD):
                    nc.gpsimd.dma_start(xT[k][:, 512:1024], xt_ext[1, k])
                for k in range(KD):
                    nc.scalar.dma_start(wv_bf[k][:], wv_ext[k * PT:(k + 1) * PT, :])
                for k in range(KD):
                    nc.sync.dma_start(xT[k][:, 1024:1536], xt_ext[2, k])
                for k in range(KD):
                    nc.gpsimd.dma_start(xT[k][:, 1536:2048], xt_ext[3, k])
                for k in range(KD):
                    nc.scalar.dma_start(
                        wo_bf[k][:], wout_ext[k * PT:(k + 1) * PT, :]
                    )
                def vproj_half(s, half):
                    # v projection for head pairs {2*half, 2*half+1} only
                    pv = psP.tile([PT, 256], F32, name="pv", tag="psP")
                    for k in range(KD):
                        nc.tensor.matmul(
                            pv[:],
                            xT[k][:, s * PT:(s + 1) * PT],
                            wv_bf[k][:, half * 256:(half + 1) * 256],
                            start=(k == 0), stop=(k == KD - 1),
                        )
                    nc.vector.tensor_copy(
                        vsb[s][:, 4 * half:4 * half + 4, 0:64],
                        pv[:].rearrange("p (h e) -> p h e", h=4),
                    )

                def qkproj_ch(m, ch, which):
                    w_bf, dstT = ((wq_bf, qT), (wk_bf, kT))[which]
                    ph = psP.tile([PT, 512], F32, name="ph", tag="psP")
                    for k in range(KD):
                        nc.tensor.matmul(
                            ph[:],
                            w_bf[k][:, m * PT:(m + 1) * PT],
                            xT[k][:, ch * 512:(ch + 1) * 512],
                            start=(k == 0), stop=(k == KD - 1),
                        )
                    nc.vector.tensor_copy(
                        dstT[m][:, ch * 512:(ch + 1) * 512], ph[:]
                    )

                def ag_full(p):
                    nc.sync.dma_start(ag_in[p][:], attoutT[p][:])
                    nc.gpsimd.collective_compute(
                        "AllGather", mybir.AluOpType.bypass,
                        replica_groups=RG,
                        ins=[ag_in[p].opt()], outs=[ag_out[p].opt()],
                    )

                def ag_iq(p, iq):
                    cs = iq * 512
                    nc.sync.dma_start(
                        ag_cin[iq][:], attoutT[p][:, cs:cs + 512]
                    )
                    nc.gpsimd.collective_compute(
                        "AllGather", mybir.AluOpType.bypass,
                        replica_groups=RG,
                        ins=[ag_cin[iq].opt()],
                        outs=[ag_chunk[iq].opt()],
                    )

                def stage_gathered(p, last):
                    # stage + mask-select pair p's two gathered k-tiles
                    for kk in (p, p + MI):
                        half = kk // MI
                        ast = agst.tile([PT, N], BF16, name="ast", tag="ast")
                        if last:
                            for iq in range(4):
                                nc.sync.dma_start(
                                    ast[:, iq * 512:(iq + 1) * 512],
                                    ag_chunk[iq][half * PT:(half + 1) * PT, :],
                                )
                        else:
                            nc.sync.dma_start(
                                ast[:], ag_out[p][half * PT:(half + 1) * PT, :]
                            )
                        tmp = agst.tile([PT, NH], BF16, name="tmp", tag="tmp")
                        nc.vector.tensor_scalar_mul(
                            tmp[:], ast[:, 0:NH], s0_bc[:]
                        )
                        nc.vector.scalar_tensor_tensor(
                            attThalf[kk],
                            ast[:, NH:N], s1_bc[:], tmp[:],
                            op0=mybir.AluOpType.mult,
                            op1=mybir.AluOpType.add,
                        )

                def oproj_partial(plist, mrange, emit_out=False):
                    # accumulate the given pairs' inner-dim tiles into the
                    # f32 output accumulator (bias folded into the first
                    # write); one DVE add per PSUM group regardless of how
                    # many pairs are batched
                    first = 0 in plist
                    kks = [p + MI * h for p in plist for h in range(2)]
                    for m in mrange:
                        for nn in range(2):
                            pp = psP.tile([PT, 512], F32, name="pp", tag="psP")
                            for ki, kk in enumerate(kks):
                                nc.tensor.matmul(
                                    pp[:],
                                    attThalf[kk][:, m * PT:(m + 1) * PT],
                                    wo_bf[kk][:, nn * 512:(nn + 1) * 512],
                                    start=(ki == 0), stop=(ki == len(kks) - 1),
                                )
                            acc = oacc[m][nn]
                            src1 = bias_bc[nn] if first else acc
                            nc.vector.tensor_tensor(
                                acc[:], pp[:], src1[:],
                                op=mybir.AluOpType.add,
                            )
                            if emit_out:
                                nc.sync.dma_start(
                                    out_ext[m * PT:(m + 1) * PT,
                                            nn * 512:(nn + 1) * 512],
                                    acc[:],
                                )

                def stage_last_cols(cp):
                    # last pair: combine gathered chunks (cp, cp+2) into
                    # seq columns [cp*512:(cp+1)*512] of both k-tiles, so
                    # the first half needn't wait for the final AG chunk
                    for kk in (MI - 1, 2 * MI - 1):
                        half = kk // MI
                        ast = agst.tile([PT, 1024], BF16, name="astl",
                                        tag="astl")
                        nc.sync.dma_start(
                            ast[:, 0:512],
                            ag_chunk[cp][half * PT:(half + 1) * PT, :],
                        )
                        nc.sync.dma_start(
                            ast[:, 512:1024],
                            ag_chunk[cp + 2][half * PT:(half + 1) * PT, :],
                        )
                        tmp = agst.tile([PT, 512], BF16, name="tmpl",
                                        tag="tmpl")
                        nc.vector.tensor_scalar_mul(
                            tmp[:], ast[:, 0:512], s0_bc[:]
                        )
                        nc.vector.scalar_tensor_tensor(
                            attThalf[kk][:, cp * 512:(cp + 1) * 512],
                            ast[:, 512:1024], s1_bc[:], tmp[:],
                            op0=mybir.AluOpType.mult,
                            op1=mybir.AluOpType.add,
                        )

                def attention(p, items):
                    last = p == MI - 1
                    # flat software pipeline over (iq, j): v-matmuls
                    # run one step behind S/exp so the next block's
                    # score matmul never queues behind exp-gated work
                    outs = {}
                    pend = None  # (iq, j, at)
                    for it in items:
                        if it[0] == "hook":
                            it[1]()
                            continue
                        iq, j = it[1], it[2]
                        if j == 0:
                            outs[iq] = (
                                psO.tile([65, 512], F32, name="oA", tag="psO"),
                                psO.tile([65, 512], F32, name="oB", tag="psO"),
                            )
                        ps = psS.tile([PT, 1024], F32, name="ps", tag="psS")
                        nc.tensor.matmul(
                            ps[:, 0:512],
                            kT[p][0:64, j * PT:(j + 1) * PT],
                            qT[p][0:64, iq * 512:(iq + 1) * 512],
                            start=True, stop=True,
                            tile_position=(0, 0),
                        )
                        nc.tensor.matmul(
                            ps[:, 512:1024],
                            kT[p][64:128, j * PT:(j + 1) * PT],
                            qT[p][64:128, iq * 512:(iq + 1) * 512],
                            start=True, stop=True,
                            tile_position=(64, 0),
                        )
                        at = attnp.tile([PT, 1024], BF16, name="at", tag="at")
                        nc.scalar.activation(
                            at[:], ps[:], mybir.ActivationFunctionType.Exp,
                            scale=SCALE,
                        )
                        if pend is not None:
                            self_emit_vmm(p, outs, *pend)
                            if pend[1] == MS - 1:
                                self_finalize(p, outs, pend[0], last)
                        pend = (iq, j, at)
                    self_emit_vmm(p, outs, *pend)
                    self_finalize(p, outs, pend[0], last)
                    if not last:
                        ag_full(p)

                def self_emit_vmm(p, outs, iq, j, at):
                    oA, oB = outs[iq]
                    nc.tensor.matmul(
                        oA[:], vsb[j][:, 2 * p, 0:65], at[:, 0:512],
                        start=(j == 0), stop=(j == MS - 1),
                    )
                    nc.tensor.matmul(
                        oB[:], vsb[j][:, 2 * p + 1, 0:65], at[:, 512:1024],
                        start=(j == 0), stop=(j == MS - 1),
                    )

                def self_finalize(p, outs, iq, last):
                    dens = []
                    for hh, o in enumerate(outs[iq]):
                        seg = attoutT[p][hh * 64:(hh + 1) * 64,
                                         iq * 512:(iq + 1) * 512]
                        nc.vector.tensor_copy(seg, o[0:64, :])
                        den = finp.tile([1, 512], F32, name="den", tag="den")
                        nc.vector.tensor_copy(den[:], o[64:65, :])
                        dens.append((hh, den))
                    for hh, den in dens:
                        recip = finp.tile([1, 512], F32, name="recip",
                                          tag="recip")
                        nc.vector.reciprocal_approx_fast(recip[:], den[:])
                        bc = finp.tile([PT, 512], F32, name="bc", tag="bc")
                        nc.gpsimd.partition_broadcast(bc[:], recip[:])
                        seg = attoutT[p][hh * 64:(hh + 1) * 64,
                                         iq * 512:(iq + 1) * 512]
                        nc.vector.tensor_tensor(
                            seg, seg, bc[hh * 64:(hh + 1) * 64, :],
                            op=mybir.AluOpType.mult,
                        )
                    if last:
                        ag_iq(p, iq)

                def build_items(hook_map):
                    items = []
                    for iq in range(4):
                        for j in range(MS):
                            for fn in hook_map.get((iq, j), ()):
                                items.append(("hook", fn))
                            items.append(("step", iq, j))
                    return items

                def vp(half, s0, s1):
                    return lambda: [vproj_half(s, half) for s in range(s0, s1)]

                def qk(m, c, w):
                    return lambda: qkproj_ch(m, c, w)

                def op(plist, m0, m1):
                    return lambda: oproj_partial(plist, range(m0, m1))

                # pair 0: ch-staged start — scores/exp for iq=0 begin as
                # soon as each 512-column block of xT lands; v projection
                # for pairs 0-1 rides the lead-in. Every hook is kept under
                # ~2us of PE work so the 3-deep score pipeline never drains.
                items0 = [("hook", qk(0, 0, 0)), ("hook", qk(0, 0, 1)),
                          ("hook", vp(0, 0, 2))]
                items0 += [("step", 0, j) for j in range(0, 2)]
                items0.append(("hook", vp(0, 2, 4)))
                items0 += [("step", 0, j) for j in range(2, 4)]
                for c in range(1, 4):
                    items0.append(("hook", qk(0, c, 1)))
                    items0.append(("hook", vp(0, 4 * c, 4 * c + 2)))
                    items0 += [("step", 0, j) for j in range(4 * c, 4 * c + 2)]
                    items0.append(("hook", qk(0, c, 0)))
                    items0.append(("hook", vp(0, 4 * c + 2, 4 * c + 4)))
                    items0 += [("step", 0, j)
                               for j in range(4 * c + 2, 4 * c + 4)]
                hm0 = {(1, 3): [qk(1, 0, 0)], (1, 6): [qk(1, 0, 1)],
                       (1, 9): [qk(1, 1, 0)], (1, 12): [qk(1, 1, 1)],
                       (2, 3): [qk(1, 2, 0)], (2, 6): [qk(1, 2, 1)],
                       (2, 9): [qk(1, 3, 0)], (2, 12): [qk(1, 3, 1)]}
                for iq in range(1, 4):
                    for j in range(MS):
                        for fn in hm0.get((iq, j), ()):
                            items0.append(("hook", fn))
                        items0.append(("step", iq, j))
                attention(0, items0)

                def qk_hooks(m):
                    return {(0, 3): [qk(m, 0, 0)], (0, 9): [qk(m, 0, 1)],
                            (1, 3): [qk(m, 1, 0)], (1, 9): [qk(m, 1, 1)],
                            (2, 3): [qk(m, 2, 0)], (2, 9): [qk(m, 2, 1)],
                            (3, 3): [qk(m, 3, 0)], (3, 9): [qk(m, 3, 1)]}

                def merge(*maps):
                    out = {}
                    for mp in maps:
                        for k, v in mp.items():
                            out.setdefault(k, []).extend(v)
                    return out

                hook_maps = {
                    1: merge(qk_hooks(2),
                             {(1, 1): [lambda: stage_gathered(0, False)],
                              (1, 6): [vp(1, 0, 2)], (1, 12): [vp(1, 2, 4)],
                              (2, 0): [vp(1, 4, 6)], (2, 6): [vp(1, 6, 8)],
                              (2, 12): [vp(1, 8, 10)], (3, 0): [vp(1, 10, 12)],
                              (3, 6): [vp(1, 12, 14)],
                              (3, 12): [vp(1, 14, 16)]}),
                    2: merge(qk_hooks(3),
                             {(1, 1): [lambda: stage_gathered(1, False)],
                              (1, 6): [op([0, 1], 0, 1)],
                              (1, 12): [op([0, 1], 1, 2)],
                              (2, 0): [op([0, 1], 2, 3)],
                              (2, 6): [op([0, 1], 3, 4)],
                              (2, 12): [op([0, 1], 4, 5)],
                              (3, 0): [op([0, 1], 5, 6)],
                              (3, 6): [op([0, 1], 6, 7)],
                              (3, 12): [op([0, 1], 7, 8)]}),
                    3: {(1, 1): [lambda: stage_gathered(2, False)],
                        (1, 3): [op([2], 0, 1)], (1, 6): [op([2], 1, 2)],
                        (1, 9): [op([2], 2, 3)], (1, 12): [op([2], 3, 4)],
                        (2, 0): [op([2], 4, 5)], (2, 3): [op([2], 5, 6)],
                        (2, 6): [op([2], 6, 7)], (2, 9): [op([2], 7, 8)]},
                }
                for p in range(1, MI):
                    attention(p, build_items(hook_maps[p]))
                    if p == 1:
                        wv_stack.close()

                # tail: seq-columns 0:512 of the gathered last pair are
                # ready after AG chunks 0 and 2 — project them while the
                # final chunk is still in flight
                stage_last_cols(0)
                oproj_partial([MI - 1], range(0, 4), emit_out=True)
                # keep the PE's HAM un-throttled across the final AG wait
                for _ in range(16):
                    wps = psP.tile([PT, 512], F32, name="warm", tag="psP")
                    nc.tensor.matmul(
                        wps[:], wq_bf[0][:, 0:PT], wq_bf[1][:],
                        start=True, stop=True,
                    )
                stage_last_cols(1)
                oproj_partial([MI - 1], range(4, 8), emit_out=True)
                oacc_stack.close()
                psum_stack.close()

    nc.compile()
    return nc


def _shard_inputs(x, Wq, Wkv, Wout, bout):
    bf = ml_dtypes.bfloat16
    # xT tiled as [ch, k, 128, 512] so each on-device DMA reads one
    # contiguous 128KB block
    xt_b = [
        np.ascontiguousarray(
            np.ascontiguousarray(x[b].T).astype(bf)
            .reshape(KD, PT, 4, 512).transpose(2, 0, 1, 3)
        )
        for b in range(B)
    ]
    wout_bf = Wout.astype(bf)
    bout_f = np.ascontiguousarray(bout, dtype=np.float32)
    in_maps = []
    for c in range(N_CORES):
        b, g = c // 2, c % 2
        sel = np.zeros((1, 2), dtype=np.float32)
        sel[0, g] = 1.0
        in_maps.append({
            "xt": xt_b[b],
            "wq": np.ascontiguousarray(Wq[:, g * IN:(g + 1) * IN]).astype(bf),
            "wk": np.ascontiguousarray(Wkv[:, g * IN:(g + 1) * IN]).astype(bf),
            "wv": np.ascontiguousarray(
                Wkv[:, D + g * IN:D + (g + 1) * IN]
            ).astype(bf),
            "wout": wout_bf,
            "bout": bout_f,
            "sel": sel,
        })
    return in_maps


def kernel(x, Wq, Wkv, Wout, bout):
    global _COMPILED
    if _COMPILED is None:
        _COMPILED = build()
    nc = _COMPILED
    in_maps = _shard_inputs(
        np.asarray(x), np.asarray(Wq), np.asarray(Wkv), np.asarray(Wout),
        np.asarray(bout),
    )
    res = bass_utils.run_bass_kernel_spmd(nc, in_maps, core_ids=list(range(N_CORES)))
    out = np.empty((B, N, D), dtype=np.float32)
    for c in range(N_CORES):
        b, g = c // 2, c % 2
        out[b, g * NH:(g + 1) * NH, :] = res.results[c]["out"]
    return out


if __name__ == "__main__":
    rng = np.random.default_rng(0)
    x = rng.standard_normal((B, N, D)).astype(np.float32)
    Wq = rng.standard_normal((D, D)).astype(np.float32) * D ** -0.5
    Wkv = rng.standard_normal((D, 2 * D)).astype(np.float32) * D ** -0.5
    Wout = rng.standard_normal((D, D)).astype(np.float32) * D ** -0.5
    bout = np.zeros((D,), dtype=np.float32)
    y = kernel(x=x, Wq=Wq, Wkv=Wkv, Wout=Wout, bout=bout)
    print("out shape:", y.shape, "finite:", np.isfinite(y).all())


# revision 39
# speedup vs baseline: 1.0452x; 1.0452x over previous
"""Distributed multi-head self-attention for Trainium2 (8 NeuronCores).

Problem: b=4, n=2048, dim=1024, heads=16, dim_head=64.
  q = x@Wq; k,v = split(x@Wkv, 2); out = softmax(q k^T / 8) v; y = out@Wout + bout

Sharding: core c <-> (batch b=c//2, head-group g=c%2). Each core computes
q/k/v + attention for its batch's 8 heads (tensor-parallel columns of
Wq/Wkv). The pair (b,0)/(b,1) AllGathers the transposed bf16 attention
outputs (per head-pair, overlapped with attention compute; the last pair
streams per-i-chunk), then each core runs the output projection with the
full Wout over ITS HALF of the sequence (selected from the gathered buffer
with per-core one-hot mask inputs, since the SPMD graph is identical on all
cores). Core 2b+g emits out rows [1024g : 1024(g+1)] of batch b; the host
reassembles [4, 2048, 1024].

Host-side prep: x is pre-transposed to xT [dim, n] and converted to bf16,
weights are sliced per-core and converted to bf16 — phase 0 is pure DMA
(no on-device casts or PE transposes). A dummy 2-rank AllGather and a tiny
exp activation are issued at t=0 so the one-time collective rendezvous
(~100us measured) and the ACT table load (~2.7us) overlap with the
projection phase instead of stalling the attention pipeline.

TensorEngine math is bf16 with f32 PSUM accumulation. Softmax skips
max-subtraction (scaled scores are ~N(0,1)); exp runs on the scalar engine
(PSUM f32 in -> bf16 SBUF out, scale fused). Denominators come from a ones
column appended to v; the PSUM accumulator is released with two fast copies
and normalization (reciprocal_approx_fast + partition_broadcast + in-place
multiply) runs lazily off the critical path. Score matmuls (K=64) run two
heads concurrently via tile_position row groups. q/k projections for pair
p+1 are emitted after attention(p) and v projections inside attention(0)'s
first column loop, so the TensorEngine fills its slack while attention is
ACT(exp)-bound.
"""

from contextlib import ExitStack

import numpy as np
import ml_dtypes

import concourse.mybir as mybir
import concourse.tile as tile
from concourse import bacc, bass_utils

N_CORES = 8
B, N, D = 4, 2048, 1024
GH = 8          # heads per core
DH = 64
IN = GH * DH    # 512 inner dims per core
SCALE = DH ** -0.5
PT = 128
KD = D // PT    # 8 dim tiles
MS = N // PT    # 16 seq tiles
MI = 4          # head pairs per core
NH = N // 2     # out rows per core
F32 = mybir.dt.float32
BF16 = mybir.dt.bfloat16
RG = [[0, 1], [2, 3], [4, 5], [6, 7]]

_COMPILED = None


def build():
    nc = bacc.Bacc("TRN2", target_bir_lowering=False, debug=False, num_devices=N_CORES)

    xt_ext = nc.dram_tensor("xt", [4, KD, PT, 512], BF16, kind="ExternalInput")
    wq_ext = nc.dram_tensor("wq", [D, IN], BF16, kind="ExternalInput")
    wk_ext = nc.dram_tensor("wk", [D, IN], BF16, kind="ExternalInput")
    wv_ext = nc.dram_tensor("wv", [D, IN], BF16, kind="ExternalInput")
    wout_ext = nc.dram_tensor("wout", [D, D], BF16, kind="ExternalInput")
    bout_ext = nc.dram_tensor("bout", [D], F32, kind="ExternalInput")
    sel_ext = nc.dram_tensor("sel", [1, 2], F32, kind="ExternalInput")
    out_ext = nc.dram_tensor("out", [NH, D], F32, kind="ExternalOutput")

    with tile.TileContext(nc) as tc:
        with (
            tc.tile_pool(name="const", bufs=1) as constp,
            tc.tile_pool(name="wpool", bufs=1) as wpool,
            tc.tile_pool(name="qkv", bufs=1) as qkv,
            tc.tile_pool(name="attout", bufs=1) as attoutp,
            tc.tile_pool(name="dram", bufs=1, space="DRAM") as dram,
        ):
            # ---- t=0 warmups: collective rendezvous + exp table load ----
            wa_in = dram.tile([1, 16], BF16, name="wa_in")
            wa_out = dram.tile([2, 16], BF16, name="wa_out")
            wa_sb = constp.tile([1, 16], BF16)
            nc.gpsimd.memset(wa_sb[:], 0.0)
            nc.sync.dma_start(wa_in[:], wa_sb[:])
            nc.gpsimd.collective_compute(
                "AllGather", mybir.AluOpType.bypass,
                replica_groups=RG,
                ins=[wa_in.opt()], outs=[wa_out.opt()],
            )
            wex_in = constp.tile([1, 16], F32)
            nc.gpsimd.memset(wex_in[:], 0.0)
            wex_out = constp.tile([1, 16], BF16)
            nc.scalar.activation(
                wex_out[:], wex_in[:], mybir.ActivationFunctionType.Exp, scale=1.0
            )

            bias_row = constp.tile([1, D], F32)
            nc.sync.dma_start(bias_row[:], bout_ext[None, :])
            bias_bc = [constp.tile([PT, 512], F32, name=f"bias_bc{nn}")
                       for nn in range(2)]
            for nn in range(2):
                nc.gpsimd.partition_broadcast(
                    bias_bc[nn][:], bias_row[:, nn * 512:(nn + 1) * 512]
                )
            sel_row = constp.tile([1, 2], F32)
            nc.sync.dma_start(sel_row[:], sel_ext[:])
            s0_bc = constp.tile([PT, 1], F32)
            s1_bc = constp.tile([PT, 1], F32)
            nc.gpsimd.partition_broadcast(s0_bc[:], sel_row[:, 0:1])
            nc.gpsimd.partition_broadcast(s1_bc[:], sel_row[:, 1:2])

            wq_bf = [wpool.tile([PT, IN], BF16, name=f"wq_bf{k}") for k in range(KD)]
            wk_bf = [wpool.tile([PT, IN], BF16, name=f"wk_bf{k}") for k in range(KD)]
            wo_bf = [wpool.tile([PT, D], BF16, name=f"wo_bf{k}") for k in range(KD)]
            xT = [wpool.tile([PT, N], BF16, name=f"xT{k}") for k in range(KD)]

            qT = [qkv.tile([PT, N], BF16, name=f"qT{m}") for m in range(MI)]
            kT = [qkv.tile([PT, N], BF16, name=f"kT{m}") for m in range(MI)]
            vsb = [qkv.tile([PT, GH, 66], BF16, name=f"v{s}") for s in range(MS)]

            attoutT = [attoutp.tile([PT, N], BF16, name=f"attoutT{p}") for p in range(MI)]
            # after AG(p) the attoutT data is snapshotted to DRAM; reuse the
            # tile halves for the mask-selected gathered k-tiles kk=p
            # (cols 0:NH) and kk=p+MI (cols NH:N)
            attThalf = [
                attoutT[k % MI][:, (k // MI) * NH:(k // MI + 1) * NH]
                for k in range(KD)
            ]
            ag_in = [dram.tile([PT, N], BF16, name=f"ag_in{p}") for p in range(MI - 1)]
            ag_out = [dram.tile([2 * PT, N], BF16, name=f"ag_out{p}") for p in range(MI - 1)]
            ag_chunk = [dram.tile([2 * PT, 512], BF16, name=f"ag_chunk{i}") for i in range(4)]
            ag_cin = [dram.tile([PT, 512], BF16, name=f"ag_cin{i}") for i in range(4)]

            psum_stack = ExitStack()
            with (
                tc.tile_pool(name="attn", bufs=5) as attnp,
                tc.tile_pool(name="fin", bufs=2) as finp,
                tc.tile_pool(name="agst", bufs=1) as agst,
            ):
                # the PE's lead over exp is capped at 2 steps (~2.3us) by
                # psS double-buffering, so every hook must stay under that
                # much PE work or the ACT engine bubbles
                psP = psum_stack.enter_context(
                    tc.tile_pool(name="psP", bufs=2, space="PSUM"))
                psS = psum_stack.enter_context(
                    tc.tile_pool(name="psS", bufs=2, space="PSUM"))
                psO = psum_stack.enter_context(
                    tc.tile_pool(name="psO", bufs=2, space="PSUM"))

                oacc_stack = ExitStack()
                oaccp = oacc_stack.enter_context(
                    tc.tile_pool(name="oaccp", bufs=1))
                oacc = [
                    [oaccp.tile([PT, 512], F32, name=f"oacc{m}_{nn}")
                     for nn in range(2)]
                    for m in range(NH // PT)
                ]

                wv_stack = ExitStack()
                wvp = wv_stack.enter_context(tc.tile_pool(name="wvp", bufs=1))
                wv_bf = [wvp.tile([PT, IN], BF16, name=f"wv_bf{k}")
                         for k in range(KD)]

                # ones columns for the softmax denominators (no data deps)
                for s in range(MS):
                    nc.gpsimd.memset(vsb[s][:, :, 64:65], 1.0)

                # ---- phase 0 DMAs. Triggers cost ~0.6us each on an engine
                # queue, so spread them across the engines that are idle
                # during the load (xt is host-tiled: contiguous blocks) ----
                for k in range(KD):
                    nc.sync.dma_start(wq_bf[k][:], wq_ext[k * PT:(k + 1) * PT, :])
                for k in range(KD):
                    nc.sync.dma_start(wk_bf[k][:], wk_ext[k * PT:(k + 1) * PT, :])
                for ch in range(4):
                    for k in range(KD):
                        nc.sync.dma_start(
                            xT[k][:, ch * 512:(ch + 1) * 512],
                            xt_ext[ch, k],
                        )
                    if ch == 0:
                        for k in range(KD):
                            nc.sync.dma_start(
                                wv_bf[k][:], wv_ext[k * PT:(k + 1) * PT, :]
                            )
                for k in range(KD):
                    nc.sync.dma_start(
                        wo_bf[k][:], wout_ext[k * PT:(k + 1) * PT, :]
                    )
                def vproj_half(s, half):
                    # v projection for head pairs {2*half, 2*half+1} only
                    pv = psP.tile([PT, 256], F32, name="pv", tag="psP")
                    for k in range(KD):
                        nc.tensor.matmul(
                            pv[:],
                            xT[k][:, s * PT:(s + 1) * PT],
                            wv_bf[k][:, half * 256:(half + 1) * 256],
                            start=(k == 0), stop=(k == KD - 1),
                        )
                    nc.vector.tensor_copy(
                        vsb[s][:, 4 * half:4 * half + 4, 0:64],
                        pv[:].rearrange("p (h e) -> p h e", h=4),
                    )

                def qkproj_ch(m, ch, which):
                    w_bf, dstT = ((wq_bf, qT), (wk_bf, kT))[which]
                    ph = psP.tile([PT, 512], F32, name="ph", tag="psP")
                    for k in range(KD):
                        nc.tensor.matmul(
                            ph[:],
                            w_bf[k][:, m * PT:(m + 1) * PT],
                            xT[k][:, ch * 512:(ch + 1) * 512],
                            start=(k == 0), stop=(k == KD - 1),
                        )
                    nc.vector.tensor_copy(
                        dstT[m][:, ch * 512:(ch + 1) * 512], ph[:]
                    )

                def ag_full(p):
                    nc.sync.dma_start(ag_in[p][:], attoutT[p][:])
                    nc.gpsimd.collective_compute(
                        "AllGather", mybir.AluOpType.bypass,
                        replica_groups=RG,
                        ins=[ag_in[p].opt()], outs=[ag_out[p].opt()],
                    )

                def ag_iq(p, iq):
                    cs = iq * 512
                    nc.sync.dma_start(
                        ag_cin[iq][:], attoutT[p][:, cs:cs + 512]
                    )
                    nc.gpsimd.collective_compute(
                        "AllGather", mybir.AluOpType.bypass,
                        replica_groups=RG,
                        ins=[ag_cin[iq].opt()],
                        outs=[ag_chunk[iq].opt()],
                    )

                def stage_gathered(p, last):
                    # stage + mask-select pair p's two gathered k-tiles
                    for kk in (p, p + MI):
                        half = kk // MI
                        ast = agst.tile([PT, N], BF16, name="ast", tag="ast")
                        if last:
                            for iq in range(4):
                                nc.sync.dma_start(
                                    ast[:, iq * 512:(iq + 1) * 512],
                                    ag_chunk[iq][half * PT:(half + 1) * PT, :],
                                )
                        else:
                            nc.sync.dma_start(
                                ast[:], ag_out[p][half * PT:(half + 1) * PT, :]
                            )
                        tmp = agst.tile([PT, NH], BF16, name="tmp", tag="tmp")
                        nc.vector.tensor_scalar_mul(
                            tmp[:], ast[:, 0:NH], s0_bc[:]
                        )
                        nc.vector.scalar_tensor_tensor(
                            attThalf[kk],
                            ast[:, NH:N], s1_bc[:], tmp[:],
                            op0=mybir.AluOpType.mult,
                            op1=mybir.AluOpType.add,
                        )

                def oproj_partial(plist, mrange, emit_out=False):
                    # accumulate the given pairs' inner-dim tiles into the
                    # f32 output accumulator (bias folded into the first
                    # write); one DVE add per PSUM group regardless of how
                    # many pairs are batched
                    first = 0 in plist
                    kks = [p + MI * h for p in plist for h in range(2)]
                    for m in mrange:
                        for nn in range(2):
                            pp = psP.tile([PT, 512], F32, name="pp", tag="psP")
                            for ki, kk in enumerate(kks):
                                nc.tensor.matmul(
                                    pp[:],
                                    attThalf[kk][:, m * PT:(m + 1) * PT],
                                    wo_bf[kk][:, nn * 512:(nn + 1) * 512],
                                    start=(ki == 0), stop=(ki == len(kks) - 1),
                                )
                            acc = oacc[m][nn]
                            src1 = bias_bc[nn] if first else acc
                            nc.vector.tensor_tensor(
                                acc[:], pp[:], src1[:],
                                op=mybir.AluOpType.add,
                            )
                            if emit_out:
                                nc.sync.dma_start(
                                    out_ext[m * PT:(m + 1) * PT,
                                            nn * 512:(nn + 1) * 512],
                                    acc[:],
                                )

                def stage_last_cols(cp):
                    # last pair: combine gathered chunks (cp, cp+2) into
                    # seq columns [cp*512:(cp+1)*512] of both k-tiles, so
                    # the first half needn't wait for the final AG chunk
                    for kk in (MI - 1, 2 * MI - 1):
                        half = kk // MI
                        ast = agst.tile([PT, 1024], BF16, name="astl",
                                        tag="astl")
                        nc.sync.dma_start(
                            ast[:, 0:512],
                            ag_chunk[cp][half * PT:(half + 1) * PT, :],
                        )
                        nc.sync.dma_start(
                            ast[:, 512:1024],
                            ag_chunk[cp + 2][half * PT:(half + 1) * PT, :],
                        )
                        tmp = agst.tile([PT, 512], BF16, name="tmpl",
                                        tag="tmpl")
                        nc.vector.tensor_scalar_mul(
                            tmp[:], ast[:, 0:512], s0_bc[:]
                        )
                        nc.vector.scalar_tensor_tensor(
                            attThalf[kk][:, cp * 512:(cp + 1) * 512],
                            ast[:, 512:1024], s1_bc[:], tmp[:],
                            op0=mybir.AluOpType.mult,
                            op1=mybir.AluOpType.add,
                        )

                def attention(p, items):
                    last = p == MI - 1
                    # flat software pipeline over (iq, j): v-matmuls
                    # run one step behind S/exp so the next block's
                    # score matmul never queues behind exp-gated work
                    outs = {}
                    pend = None  # (iq, j, at)
                    for it in items:
                        if it[0] == "hook":
                            it[1]()
                            continue
                        iq, j = it[1], it[2]
                        if j == 0:
                            outs[iq] = (
                                psO.tile([65, 512], F32, name="oA", tag="psO"),
                                psO.tile([65, 512], F32, name="oB", tag="psO"),
                            )
                        ps = psS.tile([PT, 1024], F32, name="ps", tag="psS")
                        nc.tensor.matmul(
                            ps[:, 0:512],
                            kT[p][0:64, j * PT:(j + 1) * PT],
                            qT[p][0:64, iq * 512:(iq + 1) * 512],
                            start=True, stop=True,
                            tile_position=(0, 0),
                        )
                        nc.tensor.matmul(
                            ps[:, 512:1024],
                            kT[p][64:128, j * PT:(j + 1) * PT],
                            qT[p][64:128, iq * 512:(iq + 1) * 512],
                            start=True, stop=True,
                            tile_position=(64, 0),
                        )
                        at = attnp.tile([PT, 1024], BF16, name="at", tag="at")
                        nc.scalar.activation(
                            at[:], ps[:], mybir.ActivationFunctionType.Exp,
                            scale=SCALE,
                        )
                        if pend is not None:
                            self_emit_vmm(p, outs, *pend)
                            if pend[1] == MS - 1:
                                self_finalize(p, outs, pend[0], last)
                        pend = (iq, j, at)
                    self_emit_vmm(p, outs, *pend)
                    self_finalize(p, outs, pend[0], last)
                    if not last:
                        ag_full(p)

                def self_emit_vmm(p, outs, iq, j, at):
                    oA, oB = outs[iq]
                    nc.tensor.matmul(
                        oA[:], vsb[j][:, 2 * p, 0:65], at[:, 0:512],
                        start=(j == 0), stop=(j == MS - 1),
                    )
                    nc.tensor.matmul(
                        oB[:], vsb[j][:, 2 * p + 1, 0:65], at[:, 512:1024],
                        start=(j == 0), stop=(j == MS - 1),
                    )

                def self_finalize(p, outs, iq, last):
                    dens = []
                    for hh, o in enumerate(outs[iq]):
                        seg = attoutT[p][hh * 64:(hh + 1) * 64,
                                         iq * 512:(iq + 1) * 512]
                        nc.vector.tensor_copy(seg, o[0:64, :])
                        den = finp.tile([1, 512], F32, name="den", tag="den")
                        nc.vector.tensor_copy(den[:], o[64:65, :])
                        dens.append((hh, den))
                    for hh, den in dens:
                        recip = finp.tile([1, 512], F32, name="recip",
                                          tag="recip")
                        nc.vector.reciprocal_approx_fast(recip[:], den[:])
                        bc = finp.tile([PT, 512], F32, name="bc", tag="bc")
                        nc.gpsimd.partition_broadcast(bc[:], recip[:])
                        seg = attoutT[p][hh * 64:(hh + 1) * 64,
                                         iq * 512:(iq + 1) * 512]
                        nc.vector.tensor_tensor(
                            seg, seg, bc[hh * 64:(hh + 1) * 64, :],
                            op=mybir.AluOpType.mult,
                        )
                    if last:
                        ag_iq(p, iq)

                def build_items(hook_map):
                    items = []
                    for iq in range(4):
                        for j in range(MS):
                            for fn in hook_map.get((iq, j), ()):
                                items.append(("hook", fn))
                            items.append(("step", iq, j))
                    return items

                def vp(half, s0, s1):
                    return lambda: [vproj_half(s, half) for s in range(s0, s1)]

                def qk(m, c, w):
                    return lambda: qkproj_ch(m, c, w)

                def op(plist, m0, m1):
                    return lambda: oproj_partial(plist, range(m0, m1))

                # pair 0: ch-staged start — scores/exp for iq=0 begin as
                # soon as each 512-column block of xT lands; v projection
                # for pairs 0-1 rides the lead-in. Every hook is kept under
                # ~2us of PE work so the 3-deep score pipeline never drains.
                items0 = [("hook", qk(0, 0, 0)), ("hook", qk(0, 0, 1)),
                          ("hook", vp(0, 0, 2))]
                items0 += [("step", 0, j) for j in range(0, 2)]
                items0.append(("hook", vp(0, 2, 4)))
                items0 += [("step", 0, j) for j in range(2, 4)]
                for c in range(1, 4):
                    items0.append(("hook", qk(0, c, 1)))
                    items0.append(("hook", vp(0, 4 * c, 4 * c + 2)))
                    items0 += [("step", 0, j) for j in range(4 * c, 4 * c + 2)]
                    items0.append(("hook", qk(0, c, 0)))
                    items0.append(("hook", vp(0, 4 * c + 2, 4 * c + 4)))
                    items0 += [("step", 0, j)
                               for j in range(4 * c + 2, 4 * c + 4)]
                hm0 = {(1, 3): [qk(1, 0, 0)], (1, 6): [qk(1, 0, 1)],
                       (1, 9): [qk(1, 1, 0)], (1, 12): [qk(1, 1, 1)],
                       (2, 3): [qk(1, 2, 0)], (2, 6): [qk(1, 2, 1)],
                       (2, 9): [qk(1, 3, 0)], (2, 12): [qk(1, 3, 1)]}
                for iq in range(1, 4):
                    for j in range(MS):
                        for fn in hm0.get((iq, j), ()):
                            items0.append(("hook", fn))
                        items0.append(("step", iq, j))
                attention(0, items0)

                def qk_hooks(m):
                    return {(0, 3): [qk(m, 0, 0)], (0, 9): [qk(m, 0, 1)],
                            (1, 3): [qk(m, 1, 0)], (1, 9): [qk(m, 1, 1)],
                            (2, 3): [qk(m, 2, 0)], (2, 9): [qk(m, 2, 1)],
                            (3, 3): [qk(m, 3, 0)], (3, 9): [qk(m, 3, 1)]}

                def merge(*maps):
                    out = {}
                    for mp in maps:
                        for k, v in mp.items():
                            out.setdefault(k, []).extend(v)
                    return out

                hook_maps = {
                    1: merge(qk_hooks(2),
                             {(1, 1): [lambda: stage_gathered(0, False)],
                              (1, 6): [vp(1, 0, 2)], (1, 12): [vp(1, 2, 4)],
                              (2, 0): [vp(1, 4, 6)], (2, 6): [vp(1, 6, 8)],
                              (2, 12): [vp(1, 8, 10)], (3, 0): [vp(1, 10, 12)],
                              (3, 6): [vp(1, 12, 14)],
                              (3, 12): [vp(1, 14, 16)]}),
                    2: merge(qk_hooks(3),
                             {(1, 1): [lambda: stage_gathered(1, False)],
                              (1, 6): [op([0, 1], 0, 1)],
                              (1, 12): [op([0, 1], 1, 2)],
                              (2, 0): [op([0, 1], 2, 3)],
                              (2, 6): [op([0, 1], 3, 4)],
                              (2, 12): [op([0, 1], 4, 5)],
                              (3, 0): [op([0, 1], 5, 6)],
                              (3, 6): [op([0, 1], 6, 7)],
                              (3, 12): [op([0, 1], 7, 8)]}),
                    3: {(1, 1): [lambda: stage_gathered(2, False)],
                        (1, 3): [op([2], 0, 1)], (1, 6): [op([2], 1, 2)],
                        (1, 9): [op([2], 2, 3)], (1, 12): [op([2], 3, 4)],
                        (2, 0): [op([2], 4, 5)], (2, 3): [op([2], 5, 6)],
                        (2, 6): [op([2], 6, 7)], (2, 9): [op([2], 7, 8)]},
                }
                for p in range(1, MI):
                    attention(p, build_items(hook_maps[p]))
                    if p == 1:
                        wv_stack.close()

                # tail: seq-columns 0:512 of the gathered last pair are
                # ready after AG chunks 0 and 2 — project them while the
                # final chunk is still in flight
                stage_last_cols(0)
                oproj_partial([MI - 1], range(0, 4), emit_out=True)
                # keep the PE's HAM un-throttled across the final AG wait
                for _ in range(16):
                    wps = psP.tile([PT, 512], F32, name="warm", tag="psP")
                    nc.tensor.matmul(
                        wps[:], wq_bf[0][:, 0:PT], wq_bf[1][:],
                        start=True, stop=True,
                    )
                stage_last_cols(1)
                oproj_partial([MI - 1], range(4, 8), emit_out=True)
                oacc_stack.close()
                psum_stack.close()

    nc.compile()
    return nc


def _shard_inputs(x, Wq, Wkv, Wout, bout):
    bf = ml_dtypes.bfloat16
    # xT tiled as [ch, k, 128, 512] so each on-device DMA reads one
    # contiguous 128KB block
    xt_b = [
        np.ascontiguousarray(
            np.ascontiguousarray(x[b].T).astype(bf)
            .reshape(KD, PT, 4, 512).transpose(2, 0, 1, 3)
        )
        for b in range(B)
    ]
    wout_bf = Wout.astype(bf)
    bout_f = np.ascontiguousarray(bout, dtype=np.float32)
    in_maps = []
    for c in range(N_CORES):
        b, g = c // 2, c % 2
        sel = np.zeros((1, 2), dtype=np.float32)
        sel[0, g] = 1.0
        in_maps.append({
            "xt": xt_b[b],
            "wq": np.ascontiguousarray(Wq[:, g * IN:(g + 1) * IN]).astype(bf),
            "wk": np.ascontiguousarray(Wkv[:, g * IN:(g + 1) * IN]).astype(bf),
            "wv": np.ascontiguousarray(
                Wkv[:, D + g * IN:D + (g + 1) * IN]
            ).astype(bf),
            "wout": wout_bf,
            "bout": bout_f,
            "sel": sel,
        })
    return in_maps


def kernel(x, Wq, Wkv, Wout, bout):
    global _COMPILED
    if _COMPILED is None:
        _COMPILED = build()
    nc = _COMPILED
    in_maps = _shard_inputs(
        np.asarray(x), np.asarray(Wq), np.asarray(Wkv), np.asarray(Wout),
        np.asarray(bout),
    )
    res = bass_utils.run_bass_kernel_spmd(nc, in_maps, core_ids=list(range(N_CORES)))
    out = np.empty((B, N, D), dtype=np.float32)
    for c in range(N_CORES):
        b, g = c // 2, c % 2
        out[b, g * NH:(g + 1) * NH, :] = res.results[c]["out"]
    return out


if __name__ == "__main__":
    rng = np.random.default_rng(0)
    x = rng.standard_normal((B, N, D)).astype(np.float32)
    Wq = rng.standard_normal((D, D)).astype(np.float32) * D ** -0.5
    Wkv = rng.standard_normal((D, 2 * D)).astype(np.float32) * D ** -0.5
    Wout = rng.standard_normal((D, D)).astype(np.float32) * D ** -0.5
    bout = np.zeros((D,), dtype=np.float32)
    y = kernel(x=x, Wq=Wq, Wkv=Wkv, Wout=Wout, bout=bout)
    print("out shape:", y.shape, "finite:", np.isfinite(y).all())
